# revision 9
# baseline (speedup 1.0000x reference)
"""GreedySampler kernel for 8 Trainium2 NeuronCores.

fp8 screen on device + exact host rescore of near-max candidates
(argmax(softmax(log(...))) = argmax(logits); fp8 logit error <=0.43
unscaled vs DELTA=2.0, so quantization only shortlists candidates).

v2: operand-swapped PE schedule. The padded hidden states (208 jobs,
two stationary groups of M=128/80) are the LDWEIGHTS operand; the W
vocab columns are the moving data. Per (kk, group) ONE DoubleRow
matmul streams a whole vocab block (<=2032 cols, 4 PSUM banks), so PE
time is ~52us vs the ~75us of the W-stationary schedule (800 pairs at
the N=200 issue floor) - the PE no longer trails the ~420GB/s W
stream, which is the roofline resource.

Per core (SPMD, vocab-sharded, blocks 192+2032*3 = 6288 cols):
  * W packed on host into DMA consumption order [P, bytes]; blocks
    0..2 stream on the sync HWDGE ring in kk-sliced chunks (1-4MB,
    8-32KB per-partition packets; the 16 shared DMA engines cost
    ~50ns/packet + ~30GB/s line rate, so big packets sustain >430GB/s
    aggregate), fine chunks at the stream tail to cut the PE lag.
  * The 192-col remainder block rides the scalar HWDGE ring in
    parallel with the W start; hst rides the gpsimd SWDGE ring.
  * kk-outer, group-inner accumulation: each W chunk is consumed once
    when it lands; PSUM holds both groups of one block (8 banks).
  * Eviction casts (fp32->fp8) split across DVE/ACT/GpSimd so they
    fit the inter-block DMA gap; outs ship mid-stream on gpsimd
    SWDGE, the last block on the then-idle scalar ring.

Walrus notes: instructions carrying >1 sync wait are rejected by this
build, so excess waits are split onto preceding nops; DoubleRow
operand strides must be 16B-aligned (hence jobs padded 200->208).
"""

import numpy as np
import ml_dtypes

import concourse.bass as bass
import concourse.mybir as mybir
import concourse.tile as tile
from concourse.vector_clock import ScopedClock
from concourse.bass_utils import run_bass_kernel_spmd

P = 128
N_CORES = 8
D = 4096
KK = D // 256  # 16 DoubleRow K-chunks of 256
W_SCALE = 32.0
DELTA = 2.0 * W_SCALE  # candidate margin in scaled-logit units

J = 200
JP = 208                  # jobs padded to 16-multiple for DoubleRow strides
GROUPS = [(0, 128), (128, 80)]  # (job offset, group width); jobs 200..207 pad

BLOCKS = [192, 2032, 2032, 2032]  # vocab blocks per core (2032*4B = 4 banks)
VS_EFF = sum(BLOCKS)      # 6288
V_PAD = VS_EFF * N_CORES  # 50304 >= 50257
TOTW = KK * 2 * VS_EFF    # W bytes per partition
OUT_TOT = 2 * VS_EFF      # logits bytes per partition

FP8 = mybir.dt.float8e4
F32 = mybir.dt.float32

_drain_patched = False


def _patch_tile_drain():
    """Split the tail Drain's sync waits (>1 rejected by this walrus)."""
    global _drain_patched
    if _drain_patched:
        return

    def _drain_and_barrier(self, tick_clock, wait_clock):
        nc = self.nc
        drain_inst = nc.sync.drain()
        wait_clock.add_sem_waits(
            drain_inst.ins, ScopedClock({None: tick_clock.global_clock})
        )
        si = drain_inst.ins.sync_info
        if si is not None and si.on_wait and len(si.on_wait) > 1:
            extra = list(si.on_wait[1:])
            del si.on_wait[1:]
            name2sem = {
                getattr(s, "name", None): s
                for s in self.sems.allocated().values()
            }
            for w in extra:
                nc.sync.wait_ge(name2sem[w.ant_name], w.wait_value)
        nc.all_engine_barrier()
        popped = nc._tile_sem_poison_stack.pop()
        assert popped is self._sem_poison
        nc.clear_and_free_semaphores(list(self.sems.allocated().values()))
        nc.all_engine_barrier()

    tile.TileContext._drain_and_barrier = _drain_and_barrier
    _drain_patched = True


def _split_excess_waits(nc, limit=1):
    """Move all but `limit` sync waits of every instruction onto nops
    inserted immediately before it on the same engine queue."""
    fn = nc.m.functions[0]
    for bb in fn.blocks:
        if not any(
            getattr(i, "sync_info", None) is not None
            and i.sync_info.on_wait
            and len(i.sync_info.on_wait) > limit
            for i in bb.instructions
        ):
            continue
        cur = nc.cur_bb.bb if hasattr(nc.cur_bb, "bb") else nc.cur_bb
        new_insts = []
        for inst in bb.instructions:
            si = getattr(inst, "sync_info", None)
            if si is not None and si.on_wait and len(si.on_wait) > limit:
                extra = list(si.on_wait[:-limit])
                del si.on_wait[: len(si.on_wait) - limit]
                for w in extra:
                    nop = nc.engines[inst.engine].nop(nofuse=True).ins
                    popped = cur.instructions.pop()  # nop() self-appended
                    assert popped is nop
                    nop.sync_info = mybir.SyncInfo(on_wait=[w], on_update=[])
                    new_insts.append(nop)
            new_insts.append(inst)
        bb.instructions[:] = new_insts


# kk-slice cuts per block: fine at the stream head (early PE start)
# and tail (small post-stream PE lag), halves otherwise
KK_CUTS = {
    1: [0, 2, 4, 8, 16],
    2: [0, 8, 16],
    3: [0, 8, 12, 14, 16],
}
# eviction cast column splits (16-aligned) across [vector, scalar]
# (gpsimd cannot read PSUM on TRN2)
CAST_SPLITS = [(0, 1024), (1024, 2032)]


def build_nc():
    _patch_tile_drain()

    nc = bass.Bass()
    hst = nc.dram_tensor("hst", [P, KK, 2, JP], FP8, kind="ExternalInput")
    wt = nc.dram_tensor("wt", [P, TOTW], FP8, kind="ExternalInput")
    lg = nc.dram_tensor("lg", [P, OUT_TOT], FP8, kind="ExternalOutput")

    woff = [KK * 2 * sum(BLOCKS[:b]) for b in range(len(BLOCKS))]
    ooff = [2 * sum(BLOCKS[:b]) for b in range(len(BLOCKS))]

    with tile.TileContext(nc) as tc:
        with (
            tc.tile_pool(name="hs", bufs=1) as hs_pool,
            tc.tile_pool(name="w0", bufs=1) as w0_pool,
            tc.tile_pool(name="w", bufs=2) as w_pool,
            tc.tile_pool(name="out", bufs=2) as out_pool,
            tc.tile_pool(name="wu", bufs=1) as wu_pool,
            tc.tile_pool(name="ps", bufs=2, space=bass.MemorySpace.PSUM) as ps_pool,
        ):
            # hst on the gpsimd SWDGE ring (its completion sems live
            # outside the HWDGE lanes, so it cannot stall the W ring);
            # 3 pieces so early kk rows land first
            hst_sb = hs_pool.tile([P, KK, 2, JP], FP8)
            for sl in (slice(0, 2), slice(2, 8), slice(8, KK)):
                nc.gpsimd.dma_start(hst_sb[:, sl], hst[:, sl])

            # remainder block W on the scalar HWDGE ring, in parallel
            # with the main W stream start on the sync ring
            bw0 = BLOCKS[0]
            w192_sb = w0_pool.tile([P, KK, 2, bw0], FP8, name="w192")
            nc.scalar.dma_start(
                w192_sb[:],
                wt[:, 0:KK * 2 * bw0].rearrange(
                    "p (k t w) -> p k t w", k=KK, t=2),
            )

            # main W stream on the sync HWDGE ring
            w_sbs = {}
            for b in (1, 2, 3):
                bw = BLOCKS[b]
                w_sb = w_pool.tile([P, KK, 2, bw], FP8, name="w_sb")
                w_sbs[b] = w_sb
                cuts = KK_CUTS[b]
                for a, e in zip(cuts[:-1], cuts[1:]):
                    src = wt[:, woff[b] + a * 2 * bw: woff[b] + e * 2 * bw]
                    nc.sync.dma_start(
                        w_sb[:, a:e],
                        src.rearrange("p (k t w) -> p k t w", k=e - a, t=2),
                    )

            # PE warmup: dummy DoubleRow pairs on memset tiles fill the
            # DMA-latency window so the HAM clock gate starts ramping
            # (0.65->2.4GHz) before real work arrives
            wu_w = wu_pool.tile([P, 2, P], FP8, name="wu_w")
            wu_h = wu_pool.tile([P, 2, 512], FP8, name="wu_h")
            nc.vector.memset(wu_w[:], 0.0)
            nc.vector.memset(wu_h[:], 0.0)
            wu_ps = ps_pool.tile([P, 2048], F32, name="ps")
            for _ in range(24):
                nc.tensor.matmul(
                    wu_ps[:, :512], wu_w[:], wu_h[:],
                    start=True, stop=True,
                    perf_mode=mybir.MatmulPerfMode.DoubleRow,
                )

            for b, bw in enumerate(BLOCKS):
                w_sb = w192_sb if b == 0 else w_sbs[b]
                pss = [ps_pool.tile([P, 2048], F32, name="ps")
                       for g in range(2)]
                ot = out_pool.tile([P, 2, bw], FP8,
                                   name="ot192" if b == 0 else "ot")
                # PSUM-bank-aligned sub-matmuls (a matmul output cannot
                # cross a 2KB bank); consecutive sub-matmuls share the
                # same stationary hst chunk
                bank_cuts = list(range(0, bw, 512)) + [bw]
                for kk in range(KK):
                    for g, (goff, gw) in enumerate(GROUPS):
                        for c0, c1 in zip(bank_cuts[:-1], bank_cuts[1:]):
                            nc.tensor.matmul(
                                pss[g][:gw, c0:c1],
                                hst_sb[:, kk, :, goff:goff + gw],
                                w_sb[:, kk, :, c0:c1],
                                start=(kk == 0),
                                stop=(kk == KK - 1),
                                perf_mode=mybir.MatmulPerfMode.DoubleRow,
                            )
                # eviction casts split across DVE+ACT so they fit the
                # inter-block DMA gap (DVE alone is ~2x too slow)
                engs = [nc.vector.tensor_copy, nc.scalar.copy]
                if b == 0:
                    nc.vector.tensor_copy(ot[:, 0, :], pss[0][:, :bw])
                    nc.scalar.copy(ot[:80, 1, :], pss[1][:80, :bw])
                else:
                    for g, (goff, gw) in enumerate(GROUPS):
                        for (c0, c1), eng in zip(CAST_SPLITS, engs):
                            eng(ot[:gw, g, c0:c1], pss[g][:gw, c0:c1])
                # mid-stream outs via gpsimd SWDGE (cannot stall the W
                # rings); the last block via scalar, idle post-stream
                ring = nc.gpsimd if b < len(BLOCKS) - 1 else nc.scalar
                ring.dma_start(lg[:, ooff[b]:ooff[b] + bw], ot[:, 0, :])
                ring.dma_start(
                    lg[:80, ooff[b] + bw:ooff[b] + 2 * bw], ot[:80, 1, :])

    _split_excess_waits(nc, limit=1)
    return nc


def _pack_w(shard):
    """shard [D, VS_EFF] fp8 -> [P, TOTW] partition-major, block-major,
    contiguous in DMA consumption order."""
    parts = []
    c = 0
    for bw in BLOCKS:
        a = shard[:, c:c + bw].reshape(KK, 2, P, bw)
        parts.append(np.ascontiguousarray(
            a.transpose(2, 0, 1, 3)).reshape(P, -1))
        c += bw
    return np.concatenate(parts, axis=1)


def _decode_logits(lgbuf):
    """[P, OUT_TOT] fp8 -> [J, VS_EFF] f32."""
    res = np.empty((J, VS_EFF), np.float32)
    o = 0
    c = 0
    for bw in BLOCKS:
        arr = lgbuf[:, o:o + 2 * bw].astype(np.float32).reshape(P, 2, bw)
        res[0:128, c:c + bw] = arr[:, 0, :]
        res[128:J, c:c + bw] = arr[: J - 128, 1, :]
        o += 2 * bw
        c += bw
    return res


def _job_indices(fill_tokens_num, num_generation_jobs):
    fill = np.asarray(fill_tokens_num, dtype=np.int64)
    fill_last = np.cumsum(fill) - 1
    total_fill = int(fill.sum())
    gen = total_fill + np.arange(int(num_generation_jobs), dtype=np.int64)
    return np.concatenate([fill_last, gen])


def kernel(hidden_states, embd_weight, fill_tokens_num, num_generation_jobs):
    hs = np.asarray(hidden_states, dtype=np.float32)
    W = np.asarray(embd_weight, dtype=np.float32)
    V, Dd = W.shape

    idx = _job_indices(fill_tokens_num, num_generation_jobs)
    assert idx.size == J

    hs_sel = hs[idx]
    hs_pad = np.zeros((JP, Dd), np.float32)
    hs_pad[:J] = hs_sel
    hst_host = np.ascontiguousarray(
        hs_pad.T.reshape(KK, 2, P, JP).transpose(2, 0, 1, 3)
    ).astype(ml_dtypes.float8_e4m3)

    Wq = (W * W_SCALE).astype(ml_dtypes.float8_e4m3)
    WT_pad = np.zeros((Dd, V_PAD), dtype=ml_dtypes.float8_e4m3)
    WT_pad[:, :V] = Wq.T
    shards = [
        _pack_w(WT_pad[:, i * VS_EFF:(i + 1) * VS_EFF]) for i in range(N_CORES)
    ]

    nc = build_nc()
    kernel.last_nc = nc
    kernel.last_in_maps = [
        {"hst": hst_host, "wt": shards[i]} for i in range(N_CORES)
    ]
    res = run_bass_kernel_spmd(
        nc, kernel.last_in_maps, core_ids=list(range(N_CORES))
    )
    kernel.last_results = res

    logits = np.concatenate(
        [_decode_logits(res.results[i]["lg"]) for i in range(N_CORES)],
        axis=1,
    )[:, :V]
    logits = np.where(np.isnan(logits), np.inf, logits)

    m = logits.max(axis=1, keepdims=True)
    rows, cols = np.nonzero(logits >= m - DELTA)
    exact = np.einsum(
        "ij,ij->i", hs_sel[rows].astype(np.float64), W[cols].astype(np.float64)
    )
    ids = np.zeros(J, dtype=np.int64)
    best = np.full(J, -np.inf)
    for r, c, s in zip(rows, cols, exact):
        if s > best[r]:
            best[r] = s
            ids[r] = c
    return ids.astype(np.int32)


# revision 11
# speedup vs baseline: 1.1061x; 1.1061x over previous
"""GreedySampler kernel for 8 Trainium2 NeuronCores.

fp8 screen on device + exact host rescore of near-max candidates
(argmax(softmax(log(...))) = argmax(logits); fp8 logit error <=0.43
unscaled vs DELTA=2.0, so quantization only shortlists candidates).

v3. HW model (measured): a DoubleRow fp8 MATMUL streams 1 moving
column/cycle (206c for N=200 incl. pipeline gap; the PE fp8 peak is
128x256 MACs/cycle = 157TF/s), and LDWEIGHTS overlaps the previous
MATMUL almost fully. So the W-stationary schedule (one [K=256,M<=128]
W tile per pair, all 200 jobs moving) is at the PE MAC roofline:
ceil(6288/128)=50 tiles x 16 kk = 800 pairs ~= 69us. That exceeds the
~62us W DMA stream (~420GB/s on the sync HWDGE ring), so the kernel
is PE-bound: every PE stall and every half-clock cycle is wall time.

Per core (SPMD, vocab-sharded, groups 384+640x9+144 = 6288 cols):
  * Host packs the W shard into SBUF consumption order as one
    [P, bytes] partition-major tensor (multi-KB DMA descriptors; the
    16 shared DMA engines cost ~50ns/packet + ~30GB/s line rate).
  * All W on the sync HWDGE ring, kk-sliced fine at the stream head
    (the v1 baseline's 3.5us PE stall at t=13.5us waiting for group0
    kk8-15 also reset the HAM clock ramp to half speed until 22.7us;
    fine head chunks remove both penalties).
  * hst kk0-3 rides the sync ring first (lands ~9us, before the
    first real pair); the rest rides the gpsimd SWDGE ring whose
    completion sems cannot stall the W ring's 8-lane round-robin.
  * Short warmup (16 dummy pairs) starts the HAM clock ramp
    (0.65->2.4GHz) during the hst/W landing window.
  * kk-outer accumulation, one PSUM bank per 128-col tile, 8-bank
    rotation; mid-stream evictions on DVE only, outs on gpsimd SWDGE;
    the narrow 144-col last group splits DVE/ACT and ships on the
    then-idle scalar ring, cutting the post-PE tail to ~3us.

Walrus notes: instructions carrying >1 sync wait are rejected by this
build, so excess waits are split onto preceding nops; DoubleRow lhsT
slice widths must be 16B-aligned (128/16-wide subs only).
"""

import numpy as np
import ml_dtypes

import concourse.bass as bass
import concourse.mybir as mybir
import concourse.tile as tile
from concourse.vector_clock import ScopedClock
from concourse.bass_utils import run_bass_kernel_spmd

P = 128
N_CORES = 8
D = 4096
KK = D // 256  # 16 DoubleRow K-chunks of 256
W_SCALE = 32.0
DELTA = 2.0 * W_SCALE  # candidate margin in scaled-logit units

J = 200
VGS = [384] + [640] * 9 + [144]  # vocab-group widths per core
VS_EFF = sum(VGS)         # 6288
V_PAD = VS_EFF * N_CORES  # 50304 >= 50257

# kk-slice DMA cuts per group index: fine at the stream head
W_CUTS = {0: [0, 2, 4, 8, 16], 1: [0, 4, 8, 16], 2: [0, 8, 16]}
W_CUTS_DEFAULT = [0, 16]
N_WARMUP = 16

FP8 = mybir.dt.float8e4
F32 = mybir.dt.float32

_drain_patched = False


def _patch_tile_drain():
    """Split the tail Drain's sync waits (>1 rejected by this walrus)."""
    global _drain_patched
    if _drain_patched:
        return

    def _drain_and_barrier(self, tick_clock, wait_clock):
        nc = self.nc
        drain_inst = nc.sync.drain()
        wait_clock.add_sem_waits(
            drain_inst.ins, ScopedClock({None: tick_clock.global_clock})
        )
        si = drain_inst.ins.sync_info
        if si is not None and si.on_wait and len(si.on_wait) > 1:
            extra = list(si.on_wait[1:])
            del si.on_wait[1:]
            name2sem = {
                getattr(s, "name", None): s
                for s in self.sems.allocated().values()
            }
            for w in extra:
                nc.sync.wait_ge(name2sem[w.ant_name], w.wait_value)
        nc.all_engine_barrier()
        popped = nc._tile_sem_poison_stack.pop()
        assert popped is self._sem_poison
        nc.clear_and_free_semaphores(list(self.sems.allocated().values()))
        nc.all_engine_barrier()

    tile.TileContext._drain_and_barrier = _drain_and_barrier
    _drain_patched = True


def _split_excess_waits(nc, limit=1):
    """Move all but `limit` sync waits of every instruction onto nops
    inserted immediately before it on the same engine queue."""
    fn = nc.m.functions[0]
    for bb in fn.blocks:
        if not any(
            getattr(i, "sync_info", None) is not None
            and i.sync_info.on_wait
            and len(i.sync_info.on_wait) > limit
            for i in bb.instructions
        ):
            continue
        cur = nc.cur_bb.bb if hasattr(nc.cur_bb, "bb") else nc.cur_bb
        new_insts = []
        for inst in bb.instructions:
            si = getattr(inst, "sync_info", None)
            if si is not None and si.on_wait and len(si.on_wait) > limit:
                extra = list(si.on_wait[:-limit])
                del si.on_wait[: len(si.on_wait) - limit]
                for w in extra:
                    nop = nc.engines[inst.engine].nop(nofuse=True).ins
                    popped = cur.instructions.pop()  # nop() self-appended
                    assert popped is nop
                    nop.sync_info = mybir.SyncInfo(on_wait=[w], on_update=[])
                    new_insts.append(nop)
            new_insts.append(inst)
        bb.instructions[:] = new_insts


def _sub_widths(w):
    subs = [P] * (w // P)
    if w % P:
        subs.append(w % P)
    return subs


NSUBS = [len(_sub_widths(w)) for w in VGS]
OUT_TOT = sum(NSUBS) * J


def build_nc():
    _patch_tile_drain()

    nc = bass.Bass()
    hst = nc.dram_tensor("hst", [P, KK, 2, J], FP8, kind="ExternalInput")
    wt = nc.dram_tensor("wt", [P, KK * 2 * VS_EFF], FP8, kind="ExternalInput")
    lg = nc.dram_tensor("lg", [P, OUT_TOT], FP8, kind="ExternalOutput")

    with tile.TileContext(nc) as tc:
        with (
            tc.tile_pool(name="hs", bufs=1) as hs_pool,
            tc.tile_pool(name="w", bufs=6) as w_pool,
            tc.tile_pool(name="out", bufs=4) as out_pool,
            tc.tile_pool(name="ps", bufs=8, space=bass.MemorySpace.PSUM) as ps_pool,
        ):
            # hst kk0-3 on the sync HWDGE ring FIRST (lands ~9us, just
            # ahead of the first real pair); the rest on gpsimd SWDGE
            hst_sb = hs_pool.tile([P, KK, 2, J], FP8)
            nc.sync.dma_start(hst_sb[:, 0:2], hst[:, 0:2])
            nc.sync.dma_start(hst_sb[:, 2:4], hst[:, 2:4])
            for sl in (slice(4, 10), slice(10, KK)):
                nc.gpsimd.dma_start(hst_sb[:, sl], hst[:, sl])

            # W stream on the sync ring, strict consumption order
            w_sbs = []
            woff = 0
            for vg, wv in enumerate(VGS):
                w_sb = w_pool.tile([P, KK, 2, wv], FP8, name="w_sb")
                w_sbs.append(w_sb)
                cuts = W_CUTS.get(vg, W_CUTS_DEFAULT)
                for a, e in zip(cuts[:-1], cuts[1:]):
                    src = wt[:, woff + a * 2 * wv: woff + e * 2 * wv]
                    nc.sync.dma_start(
                        w_sb[:, a:e],
                        src.rearrange("p (k t w) -> p k t w", k=e - a, t=2),
                    )
                woff += KK * 2 * wv

            # PE warmup: dummy DoubleRow pairs start the HAM clock ramp
            # while hst/W land
            wu_w = out_pool.tile([P, 2, P], FP8, name="wu_w")
            wu_h = out_pool.tile([P, 2, J], FP8, name="wu_h")
            nc.vector.memset(wu_w[:], 0.0)
            nc.vector.memset(wu_h[:], 0.0)

            ooff = 0
            for vg, wv in enumerate(VGS):
                subs = _sub_widths(wv)
                w_sb = w_sbs[vg]
                last = vg == len(VGS) - 1
                pss = [ps_pool.tile([P, 512], F32, name="ps") for _ in subs]
                if vg == 0:
                    # complete (start+stop) dummy groups; the bank is
                    # free again before the real kk=0 accumulation
                    for _ in range(N_WARMUP):
                        nc.tensor.matmul(
                            pss[0][:, :J], wu_w[:], wu_h[:],
                            start=True, stop=True,
                            perf_mode=mybir.MatmulPerfMode.DoubleRow,
                        )
                for kk in range(KK):
                    soff = 0
                    for s, sw in enumerate(subs):
                        nc.tensor.matmul(
                            pss[s][:sw, :J],
                            w_sb[:, kk, :, soff:soff + sw],
                            hst_sb[:, kk, :, :],
                            start=(kk == 0),
                            stop=(kk == KK - 1),
                            perf_mode=mybir.MatmulPerfMode.DoubleRow,
                        )
                        soff += sw
                # evictions: DVE only mid-stream (ACT/scalar queue kept
                # clear); the post-stream last group splits DVE/ACT so
                # the tail drains in parallel
                ot = out_pool.tile([P, len(subs), J], FP8, name="ot")
                for s, sw in enumerate(subs):
                    if last and s % 2 == 1:
                        nc.scalar.copy(ot[:sw, s, :], pss[s][:sw, :J])
                    else:
                        nc.vector.tensor_copy(ot[:sw, s, :], pss[s][:sw, :J])
                # outs: gpsimd SWDGE mid-stream (its completion sems
                # cannot stall the W ring), scalar for the last group
                ring = nc.scalar if last else nc.gpsimd
                nfull = sum(1 for sw in subs if sw == P)
                if nfull == len(subs):
                    ring.dma_start(
                        lg[:, ooff:ooff + nfull * J],
                        ot[:].rearrange("p s j -> p (s j)"),
                    )
                else:
                    ring.dma_start(
                        lg[:, ooff:ooff + nfull * J],
                        ot[:, :nfull, :].rearrange("p s j -> p (s j)"),
                    )
                    sw = subs[-1]
                    ring.dma_start(
                        lg[:sw, ooff + nfull * J:ooff + (nfull + 1) * J],
                        ot[:sw, nfull, :],
                    )
                ooff += len(subs) * J

    _split_excess_waits(nc, limit=1)
    return nc


def _pack_w(shard):
    """shard [D, VS_EFF] fp8 -> [P, bytes] partition-major, vg-blocked,
    contiguous in DMA consumption order."""
    blocks = []
    off = 0
    for wv in VGS:
        a = shard[:, off:off + wv].reshape(KK, 2, P, wv)
        blocks.append(np.ascontiguousarray(
            a.transpose(2, 0, 1, 3)).reshape(P, -1))
        off += wv
    return np.concatenate(blocks, axis=1)


def _decode_logits(lgbuf):
    """[P, OUT_TOT] fp8 -> [J, VS_EFF] f32."""
    res = np.empty((J, VS_EFF), np.float32)
    o = 0
    c = 0
    arr = lgbuf.astype(np.float32)
    for vg, wv in enumerate(VGS):
        for sw in _sub_widths(wv):
            res[:, c:c + sw] = arr[:sw, o:o + J].T
            o += J
            c += sw
    return res


def _job_indices(fill_tokens_num, num_generation_jobs):
    fill = np.asarray(fill_tokens_num, dtype=np.int64)
    fill_last = np.cumsum(fill) - 1
    total_fill = int(fill.sum())
    gen = total_fill + np.arange(int(num_generation_jobs), dtype=np.int64)
    return np.concatenate([fill_last, gen])


def kernel(hidden_states, embd_weight, fill_tokens_num, num_generation_jobs):
    hs = np.asarray(hidden_states, dtype=np.float32)
    W = np.asarray(embd_weight, dtype=np.float32)
    V, Dd = W.shape

    idx = _job_indices(fill_tokens_num, num_generation_jobs)
    assert idx.size == J

    hs_sel = hs[idx]
    hst_host = np.ascontiguousarray(
        hs_sel.T.reshape(KK, 2, P, J).transpose(2, 0, 1, 3)
    ).astype(ml_dtypes.float8_e4m3)

    Wq = (W * W_SCALE).astype(ml_dtypes.float8_e4m3)
    WT_pad = np.zeros((Dd, V_PAD), dtype=ml_dtypes.float8_e4m3)
    WT_pad[:, :V] = Wq.T
    shards = [
        _pack_w(WT_pad[:, i * VS_EFF:(i + 1) * VS_EFF]) for i in range(N_CORES)
    ]

    nc = build_nc()
    kernel.last_nc = nc
    kernel.last_in_maps = [
        {"hst": hst_host, "wt": shards[i]} for i in range(N_CORES)
    ]
    res = run_bass_kernel_spmd(
        nc, kernel.last_in_maps, core_ids=list(range(N_CORES))
    )
    kernel.last_results = res

    logits = np.concatenate(
        [_decode_logits(res.results[i]["lg"]) for i in range(N_CORES)],
        axis=1,
    )[:, :V]
    logits = np.where(np.isnan(logits), np.inf, logits)

    m = logits.max(axis=1, keepdims=True)
    rows, cols = np.nonzero(logits >= m - DELTA)
    exact = np.einsum(
        "ij,ij->i", hs_sel[rows].astype(np.float64), W[cols].astype(np.float64)
    )
    ids = np.zeros(J, dtype=np.int64)
    best = np.full(J, -np.inf)
    for r, c, s in zip(rows, cols, exact):
        if s > best[r]:
            best[r] = s
            ids[r] = c
    return ids.astype(np.int32)


# revision 12
# speedup vs baseline: 1.2185x; 1.1016x over previous
"""GreedySampler kernel for 8 Trainium2 NeuronCores.

fp8 screen on device + exact host rescore of near-max candidates
(argmax(softmax(log(...))) = argmax(logits); fp8 logit error <=0.43
unscaled vs DELTA=2.0, so quantization only shortlists candidates).

Per core (SPMD, vocab-sharded, ragged 9x640+1x528 = 6288 cols):
  * Host packs the W shard into SBUF consumption order as one
    [P, bytes] partition-major tensor: all DMA chunks contiguous per
    partition (multi-KB descriptors; the naive strided layout's 512B
    descriptors cap at ~272GB/s, packed sustains ~320GB/s).
  * All W on the sync HWDGE ring in 0.5-1.3MB chunks (each dma_start
    costs ~600ns of HWDGE issue; the scalar ring starves under load;
    balanced dual-ring reaches 375GB/s but slows the PE ~20% via SBUF
    write contention - net loss).
  * hst and mid-stream output DMAs ride the gpsimd SWDGE ring, whose
    completion sems live outside the 8 round-robin HWDGE lanes, so
    late completions cannot block W DMA issue; the last group's
    output uses the then-idle scalar ring.
  * kk-outer accumulation over 5 concurrent PSUM banks (groups cannot
    share a 2KB bank); fine W chunks at the start (early PE start
    while cold) and end (small post-stream lag).
  * The 800 fp8 DoubleRow (LDWEIGHTS+MATMUL) pairs stream at
    ~86-92ns, the N=200 issue floor; fp32 PSUM accumulate, fp8 out.

Walrus notes: instructions carrying >1 sync wait are rejected by this
build, so excess waits are split onto preceding nops; DoubleRow lhsT
strides must be 16B-aligned (last group width 528, not 523).
"""

import math

import numpy as np
import ml_dtypes

import concourse.bass as bass
import concourse.mybir as mybir
import concourse.tile as tile
from concourse.vector_clock import ScopedClock
from concourse.bass_utils import run_bass_kernel_spmd

P = 128
N_CORES = 8
D = 4096
KK = D // 256  # 16 DoubleRow K-chunks of 256
W_SCALE = 32.0
DELTA = 2.0 * W_SCALE  # candidate margin in scaled-logit units

VGS = [640] * 9 + [528]   # ragged vocab-group widths per core
VS_EFF = sum(VGS)         # 6288
V_PAD = VS_EFF * N_CORES  # 50304 >= 50257

FP8 = mybir.dt.float8e4
F32 = mybir.dt.float32

_drain_patched = False


def _patch_tile_drain():
    """Split the tail Drain's sync waits (>1 rejected by this walrus)."""
    global _drain_patched
    if _drain_patched:
        return

    def _drain_and_barrier(self, tick_clock, wait_clock):
        nc = self.nc
        drain_inst = nc.sync.drain()
        wait_clock.add_sem_waits(
            drain_inst.ins, ScopedClock({None: tick_clock.global_clock})
        )
        si = drain_inst.ins.sync_info
        if si is not None and si.on_wait and len(si.on_wait) > 1:
            extra = list(si.on_wait[1:])
            del si.on_wait[1:]
            name2sem = {
                getattr(s, "name", None): s
                for s in self.sems.allocated().values()
            }
            for w in extra:
                nc.sync.wait_ge(name2sem[w.ant_name], w.wait_value)
        nc.all_engine_barrier()
        popped = nc._tile_sem_poison_stack.pop()
        assert popped is self._sem_poison
        nc.clear_and_free_semaphores(list(self.sems.allocated().values()))
        nc.all_engine_barrier()

    tile.TileContext._drain_and_barrier = _drain_and_barrier
    _drain_patched = True


def _split_excess_waits(nc, limit=1):
    """Move all but `limit` sync waits of every instruction onto nops
    inserted immediately before it on the same engine queue."""
    fn = nc.m.functions[0]
    for bb in fn.blocks:
        if not any(
            getattr(i, "sync_info", None) is not None
            and i.sync_info.on_wait
            and len(i.sync_info.on_wait) > limit
            for i in bb.instructions
        ):
            continue
        cur = nc.cur_bb.bb if hasattr(nc.cur_bb, "bb") else nc.cur_bb
        new_insts = []
        for inst in bb.instructions:
            si = getattr(inst, "sync_info", None)
            if si is not None and si.on_wait and len(si.on_wait) > limit:
                extra = list(si.on_wait[:-limit])
                del si.on_wait[: len(si.on_wait) - limit]
                for w in extra:
                    nop = nc.engines[inst.engine].nop(nofuse=True).ins
                    popped = cur.instructions.pop()  # nop() self-appended
                    assert popped is nop
                    nop.sync_info = mybir.SyncInfo(on_wait=[w], on_update=[])
                    new_insts.append(nop)
            new_insts.append(inst)
        bb.instructions[:] = new_insts


def _sub_widths(w):
    subs = [P] * (w // P)
    if w % P:
        subs.append(w % P)
    return subs


def build_nc(J, vgs=VGS):
    _patch_tile_drain()
    total = KK * 2 * sum(vgs)

    nc = bass.Bass()
    hst = nc.dram_tensor("hst", [P, KK, 2, J], FP8, kind="ExternalInput")
    wt = nc.dram_tensor("wt", [P, total], FP8, kind="ExternalInput")
    nsub_max = max(len(_sub_widths(w)) for w in vgs)
    logits_t = nc.dram_tensor("logits_t", [len(vgs), P, nsub_max * J], FP8,
                              kind="ExternalOutput")

    with tile.TileContext(nc) as tc:
        with (
            tc.tile_pool(name="hs", bufs=1) as hs_pool,
            tc.tile_pool(name="w", bufs=6) as w_pool,
            tc.tile_pool(name="out", bufs=4) as out_pool,
            tc.tile_pool(name="ps", bufs=8, space=bass.MemorySpace.PSUM) as ps_pool,
        ):
            # hst on the gpsimd SWDGE ring: off the sync ring (whose
            # serial order would delay every W byte) and off the scalar
            # ring (which HW-starves vs sync, poisoning the 8-lane DMA
            # sem round-robin). 2 pieces so early kk rows land first.
            hst_sb = hs_pool.tile([P, KK, 2, J], FP8)
            for sl in (slice(0, 2), slice(2, 8), slice(8, KK)):
                nc.gpsimd.dma_start(hst_sb[:, sl], hst[:, sl])

            # PE warmup: dummy DoubleRow pairs on memset tiles fill the
            # DMA-latency window before the first real pair, so the HAM
            # clock gate unthrottles (1.2->2.4GHz needs ~3.4us of PE
            # activity) before real work arrives
            wu_w = out_pool.tile([P, 2, P], FP8, name="wu_w")
            wu_h = out_pool.tile([P, 2, J], FP8, name="wu_h")
            nc.vector.memset(wu_w[:], 0.0)
            nc.vector.memset(wu_h[:], 0.0)

            # out-DMAs are batched: every HWDGE DMA occupies one of 8
            # round-robin completion-sem lanes, and a late-completing
            # out-DMA on a lane blocks the W DMA 8 positions later
            if len(vgs) == 10:
                ogroups = [(0, 4), (4, 4), (8, 1), (9, 1)]
            else:
                ogroups = [(v, 1) for v in range(len(vgs))]
            group_of = {}
            for gi, (a, n) in enumerate(ogroups):
                for v in range(a, a + n):
                    group_of[v] = gi
            ot = None

            nsubs = {wv: len(_sub_widths(wv)) for wv in set(vgs)}
            off = 0
            nch = 0
            for vg, wv in enumerate(vgs):
                subs = _sub_widths(wv)
                w_sb = w_pool.tile([P, KK, 2, wv], FP8, name="w_sb")
                # W chunks alternate between the two HWDGE rings (sync
                # and scalar): one ring under 8-core load sustains only
                # ~300GB/s; two rings reach ~375GB/s (HW-measured).
                # Both ring queues carry ONLY W DMAs - any PE-dependent
                # instruction there would block later DMA issues.
                # Fine chunks at the start (fast PE start) and end
                # (small post-stream lag); halves otherwise (each
                # dma_start costs ~600ns HWDGE issue time).
                if vg == 0:
                    kk_cuts = [0, 4, 8, KK]
                elif vg == len(vgs) - 1:
                    kk_cuts = [0, 8, 12, 14, KK]
                else:
                    kk_cuts = [0, 8, KK]
                for a, b in zip(kk_cuts[:-1], kk_cuts[1:]):
                    src = wt[:, off + a * 2 * wv: off + b * 2 * wv]
                    # all W on the sync ring: the scalar ring is starved
                    # under load (its chunks complete late and stall the
                    # PE), and balanced dual-ring slows the PE ~20% via
                    # SBUF write contention
                    nc.sync.dma_start(
                        w_sb[:, a:b],
                        src.rearrange("p (k t w) -> p k t w", k=b - a, t=2),
                    )
                    nch += 1

                gi = group_of[vg]
                ga, gn = ogroups[gi]
                if vg == ga:
                    ot = out_pool.tile([P, gn, nsubs[wv], J], FP8, name="ot")
                # one 2KB PSUM bank per sub: concurrent accumulation
                # groups cannot share a bank (zero region)
                pss = [ps_pool.tile([P, 512], F32, name="ps") for _ in subs]
                if vg == 0:
                    # complete (start+stop) dummy groups; the bank is
                    # free again before the real kk=0 accumulation
                    for _ in range(28):
                        nc.tensor.matmul(
                            pss[0][:, :J], wu_w[:], wu_h[:],
                            start=True, stop=True,
                            perf_mode=mybir.MatmulPerfMode.DoubleRow,
                        )
                for kk in range(KK):
                    soff = 0
                    for s, sw in enumerate(subs):
                        nc.tensor.matmul(
                            pss[s][:sw, :J],
                            w_sb[:, kk, :, soff:soff + sw],
                            hst_sb[:, kk, :, :],
                            start=(kk == 0),
                            stop=(kk == KK - 1),
                            perf_mode=mybir.MatmulPerfMode.DoubleRow,
                        )
                        soff += sw
                # fp8 copies on DVE only mid-stream (the scalar SEQ
                # must stay free for its W ring); the post-stream last
                # group splits DVE/ACT so the tail drains in parallel
                last = vg == len(vgs) - 1
                for s, sw in enumerate(subs):
                    if last and s % 2 == 1:
                        nc.scalar.copy(ot[:sw, vg - ga, s, :],
                                       pss[s][:sw, :J])
                    else:
                        nc.vector.tensor_copy(ot[:sw, vg - ga, s, :],
                                              pss[s][:sw, :J])
                if vg == ga + gn - 1:
                    # mid-stream groups ship via gpsimd (SWDGE has its
                    # own completion-sem lanes, so a late out cannot
                    # block the W rings' 8-lane round-robin); the last
                    # group ships via scalar, whose ring is free once
                    # the W stream has ended
                    nfull = sum(1 for sw in subs if sw == P)
                    if nfull == len(subs):
                        nc.gpsimd.dma_start(
                            logits_t[ga:ga + gn].rearrange("v p x -> p v x"),
                            ot[:].rearrange("p v s j -> p v (s j)"),
                        )
                    else:
                        nc.scalar.dma_start(
                            logits_t[vg, :, :nfull * J],
                            ot[:, 0, :nfull, :].rearrange("p s j -> p (s j)"),
                        )
                        sw = subs[-1]
                        nc.scalar.dma_start(
                            logits_t[vg, :sw, nfull * J:(nfull + 1) * J],
                            ot[:sw, 0, nfull, :],
                        )
                off += KK * 2 * wv

    _split_excess_waits(nc, limit=1)
    return nc


def _pack_w(shard, vgs=VGS):
    """shard [D, VS_EFF] fp8 -> [P, KK*2*VS_EFF] partition-major,
    vg-blocked, contiguous in DMA consumption order."""
    blocks = []
    off = 0
    for wv in vgs:
        a = shard[:, off:off + wv].reshape(KK, 2, P, wv)
        blocks.append(np.ascontiguousarray(
            a.transpose(2, 0, 1, 3)).reshape(P, -1))
        off += wv
    return np.concatenate(blocks, axis=1)


def _decode_logits(out, vgs, J):
    """[NVG, P, nsub_max*J] fp8 -> [VS_EFF, J] f32."""
    nvg = len(vgs)
    nsub_max = out.shape[2] // J
    res = np.empty((sum(vgs), J), np.float32)
    off = 0
    o = out.astype(np.float32).reshape(nvg, P, nsub_max, J)
    for vg, wv in enumerate(vgs):
        for s, sw in enumerate(_sub_widths(wv)):
            res[off:off + sw] = o[vg, :sw, s]
            off += sw
    return res


def _job_indices(fill_tokens_num, num_generation_jobs):
    fill = np.asarray(fill_tokens_num, dtype=np.int64)
    fill_last = np.cumsum(fill) - 1
    total_fill = int(fill.sum())
    gen = total_fill + np.arange(int(num_generation_jobs), dtype=np.int64)
    return np.concatenate([fill_last, gen])


def kernel(hidden_states, embd_weight, fill_tokens_num, num_generation_jobs):
    hs = np.asarray(hidden_states, dtype=np.float32)
    W = np.asarray(embd_weight, dtype=np.float32)
    V, Dd = W.shape

    idx = _job_indices(fill_tokens_num, num_generation_jobs)
    J = idx.size

    hs_sel = hs[idx]
    hst_host = np.ascontiguousarray(
        hs_sel.T.reshape(Dd // 256, 2, P, J).transpose(2, 0, 1, 3)
    ).astype(ml_dtypes.float8_e4m3)

    Wq = (W * W_SCALE).astype(ml_dtypes.float8_e4m3)
    WT_pad = np.zeros((Dd, V_PAD), dtype=ml_dtypes.float8_e4m3)
    WT_pad[:, :V] = Wq.T
    shards = [
        _pack_w(WT_pad[:, i * VS_EFF:(i + 1) * VS_EFF]) for i in range(N_CORES)
    ]

    nc = build_nc(J)
    kernel.last_nc = nc
    kernel.last_in_maps = [
        {"hst": hst_host, "wt": shards[i]} for i in range(N_CORES)
    ]
    res = run_bass_kernel_spmd(
        nc, kernel.last_in_maps, core_ids=list(range(N_CORES))
    )
    kernel.last_results = res

    logits = np.concatenate(
        [_decode_logits(res.results[i]["logits_t"], VGS, J)
         for i in range(N_CORES)],
        axis=0,
    ).T[:, :V]
    logits = np.where(np.isnan(logits), np.inf, logits)

    m = logits.max(axis=1, keepdims=True)
    rows, cols = np.nonzero(logits >= m - DELTA)
    exact = np.einsum(
        "ij,ij->i", hs_sel[rows].astype(np.float64), W[cols].astype(np.float64)
    )
    ids = np.zeros(J, dtype=np.int64)
    best = np.full(J, -np.inf)
    for r, c, s in zip(rows, cols, exact):
        if s > best[r]:
            best[r] = s
            ids[r] = c
    return ids.astype(np.int32)

